# revision 16
# baseline (speedup 1.0000x reference)
"""Trainium2 Bass kernel for nn_KernelGraphCalcLayer (GNN message passing).

Computation (per batch b):
    h = relu(node_feats @ weight + bias)            # (N, OUT_DIM)
    h = h.reshape(N, K, DK)
    out[n, k, d] = sum_m adj[k, n, m] * h[m, k, d]  # per-kernel dense aggregation

Sharding: batch dim (64) split across 8 NeuronCores, 8 batches per core.
No cross-device communication.

Strategy (v2): the kernel is memory-bound, so all device-side data
movement is minimized and all layout work is hoisted to the host:
  - Inputs are pre-cast to bf16 on the host (the device matmuls ran in
    bf16 already, so numerics are unchanged) -- halves HBM traffic to
    ~12.5MB/core (8MB adjT + 2MB xT + 0.5MB W + 2MB out).
  - adj is pre-transposed AND pre-packed on the host into the exact
    SBUF image [b, p, (k, c, n)] with m = c*128+p, so the PE needs NO
    on-chip transposes at all (the baseline burned ~40% of PE time on
    40 transposes + PSUM drains per batch) and every DMA has multi-KB
    contiguous per-partition runs.  x is likewise pre-transposed to
    [b, p, (ic, n)] (i = ic*128+p) so the linear's lhsT is DMA-direct.
  - Aggregation computes OT[kd, n] = sum_m h[m, kd] * adjT[m, n]:
    h slices (64 cols) are the stationary operand, adjT streams 256
    wide -- 16 matmuls/batch instead of 32, no LDWEIGHTS bloat.
    The output lands transposed+k-interleaved in PSUM; the host undoes
    the permutation for free.
  - PSUM k-placement (q=k%2 bank, s=(k//2)%2 partition-half,
    t=k//4 column-half) gives the 4 concurrently-open accumulation
    groups distinct (bank, partition-range) so start=True bit-clears
    never corrupt a pending group.  t=0 groups fully close before t=1
    groups open (PE executes in program order).
  - relu drains are split in column halves (first half feeds the t=0
    aggregation) and spread over ScalarE (h0) / DVE (h1); OT bank casts
    split DVE/ScalarE.  Stores ride SWDGE except the last two batches
    (by-then-idle HWDGE queues).
  - Per batch: 10 linear MMs (incl. 2 bias-seed MMs -- PSUM preload via
    DVE is unsafe, has_written bits) + 16 aggregation MMs.  PE ~4us,
    DMA ~3.2us per batch; both engines stay saturated; HAM stays warm.
"""

import numpy as np
import ml_dtypes

import concourse.bass as bass
import concourse.mybir as mybir
from concourse import bacc
import concourse.tile as tile
from concourse.bass_utils import run_bass_kernel_spmd

B, N, IN_DIM, OUT_DIM, K = 64, 256, 512, 512, 8
DK = OUT_DIM // K
N_CORES = 8
BPC = B // N_CORES  # batches per core

FP32 = mybir.dt.float32
BF16 = mybir.dt.bfloat16
P = 128  # SBUF partitions
NC2 = N // P       # 2 node chunks of 128
IC4 = IN_DIM // P  # 4 input-feature chunks
BF = ml_dtypes.bfloat16

_compiled = {}


def _build():
    nc = bacc.Bacc("TRN2", target_bir_lowering=False, debug=False)
    # Host-packed layouts (see module docstring):
    #   xt:  [b, p, (ic, n)]      i = ic*128+p
    #   adj: [b, p, (k, c, n)]    m = c*128+p  (pre-transposed adjacency)
    #   w:   [p, (ic, o)]         i = ic*128+p
    xt_ap = nc.dram_tensor("xt", [BPC, P, IC4 * N], BF16, kind="ExternalInput").ap()
    adj_ap = nc.dram_tensor("adjp", [BPC, P, K * NC2 * N], BF16,
                            kind="ExternalInput").ap()
    w_ap = nc.dram_tensor("w", [P, IC4 * OUT_DIM], BF16, kind="ExternalInput").ap()
    b_ap = nc.dram_tensor("bias", [OUT_DIM], BF16, kind="ExternalInput").ap()
    # OT packed: out2[b, q][p, col]: o = t*256 + s*128 + q*64 + (p%64),
    # n = col%256, with s = p//64, t = col//256 (host undoes this).
    out_ap = nc.dram_tensor("out", [BPC, 2, P, OUT_DIM], BF16,
                            kind="ExternalOutput").ap()

    PF = 3             # batches of prefetch issued ahead
    AH = NC2 * N       # adj free elems per (k-half): 4k * 2c * 256n / 2... per 4 k's
    A4 = 4 * NC2 * N   # free elems for 4 k slices

    with tile.TileContext(nc) as tc:
        with (
            tc.tile_pool(name="singles", bufs=1) as singles,
            # Pools sized so every load tile for all 8 batches is resident
            # at once (~110KB/partition total) -- load dma_start issues then
            # NEVER block on tile-free semaphores.  A blocking issue in the
            # Scalar engine's in-order queue was observed to head-of-line
            # block the next batch's relu, stalling the PE ~3.4us and
            # re-throttling HAM.
            tc.tile_pool(name="p_x", bufs=16) as p_x,
            tc.tile_pool(name="p_adj", bufs=8) as p_adj,
            tc.tile_pool(name="p_h", bufs=8) as p_h,
            tc.tile_pool(name="p_out", bufs=8) as p_out,
            tc.tile_pool(name="ps_h", bufs=4, space=bass.MemorySpace.PSUM) as ps_h,
            tc.tile_pool(name="ps_o", bufs=4, space=bass.MemorySpace.PSUM) as ps_o,
        ):
            pref = {}
            XH = IC4 * N // 2  # x half-tile free elems (ic 0,1 | ic 2,3)
            AK = NC2 * N       # adj free elems per k slice

            def prefetch(b):
                # x halves lead both HWDGE queues (linear opens each batch's
                # PE work); adj k-halves split across sync/scalar.  (A third
                # SWDGE queue was tried and dragged ALL queues down ~25% --
                # SWDGE packets round-robin on the same 16 SDMA engines.)
                xa = p_x.tile([P, XH], BF16, tag="x", name=f"xa{b}")
                nc.sync.dma_start(out=xa[:], in_=xt_ap[b, :, :XH])
                xb = p_x.tile([P, XH], BF16, tag="x", name=f"xb{b}")
                nc.scalar.dma_start(out=xb[:], in_=xt_ap[b, :, XH:])
                aS = p_adj.tile([P, 4 * AK], BF16, tag="adjS", name=f"aS{b}")
                nc.sync.dma_start(out=aS[:], in_=adj_ap[b, :, :4 * AK])
                aC = p_adj.tile([P, 4 * AK], BF16, tag="adjC", name=f"aC{b}")
                nc.scalar.dma_start(out=aC[:], in_=adj_ap[b, :, 4 * AK:])
                pref[b] = (aS, aC, xa, xb)

            # --- startup: W halves lead both queues, then batch prefetches
            w_sb = singles.tile([P, IC4 * OUT_DIM], BF16, name="w")
            nc.sync.dma_start(out=w_sb[:, :2 * OUT_DIM], in_=w_ap[:, :2 * OUT_DIM])
            nc.scalar.dma_start(out=w_sb[:, 2 * OUT_DIM:], in_=w_ap[:, 2 * OUT_DIM:])
            ones_row = singles.tile([1, P], BF16)
            nc.gpsimd.memset(ones_row[:], 1.0)
            wrow = singles.tile([1, OUT_DIM], BF16)
            nc.gpsimd.memset(wrow[:], 1.0)
            bias_c = singles.tile([1, OUT_DIM], BF16)
            nc.gpsimd.dma_start(out=bias_c[:], in_=b_ap[None, :])
            # Preload the Relu ACT table off the critical path (else the
            # first real relu pays ~1.5us of ACT_TABLE_LOAD).
            scratch = singles.tile([1, P], BF16)
            nc.scalar.activation(scratch[:], ones_row[:],
                                 mybir.ActivationFunctionType.Relu)
            # HAM warmup: dummy matmuls keep the PE busy from the moment the
            # preamble ends, so the 4096-cycle activity window un-throttles
            # the clock (1.2 -> 2.4 GHz) by the time real data arrives
            # (~3us of cold-rate 512-wide streams bridges to the first
            # linear without delaying it).
            wps = ps_o.tile([P, OUT_DIM], FP32, tag="pso", name="warm")
            wps2 = ps_o.tile([P, OUT_DIM], FP32, tag="pso", name="warm2")
            for i in range(8):
                w_t = wps if i % 2 == 0 else wps2
                nc.tensor.matmul(w_t[:], ones_row[:], wrow[:],
                                 start=True, stop=True)
            for b in range(PF):
                prefetch(b)

            for b in range(BPC):
                aS, aC, xa, xb = pref.pop(b)

                # --- linear: h[n, o] = relu(x @ W + bias), bf16, 2 n-chunks
                ph = [ps_h.tile([P, OUT_DIM], FP32, tag="psh", name=f"ph{b}_{i}")
                      for i in range(NC2)]
                h_sb = []
                for nch in range(NC2):
                    nc.tensor.matmul(ph[nch][:], ones_row[:], bias_c[:],
                                     start=True, stop=False)
                    for ic in range(IC4):
                        xt_sb = xa if ic < 2 else xb
                        o = (ic % 2) * N + nch * P
                        nc.tensor.matmul(
                            ph[nch][:], xt_sb[:, o:o + P],
                            w_sb[:, ic * OUT_DIM:(ic + 1) * OUT_DIM],
                            start=False, stop=(ic == IC4 - 1))
                    h_sb.append(p_h.tile([P, OUT_DIM], BF16, tag="h",
                                         name=f"h{b}_{nch}"))

                # relu drains on ScalarE (DVE reads PSUM ~40% slower): h0
                # whole (overlaps the nch=1 linear), h1 in column halves so
                # the first half is ready when the t=0 c=1 aggregation
                # matmuls need it.
                HO = OUT_DIM // 2
                nc.scalar.activation(h_sb[0][:], ph[0][:],
                                     mybir.ActivationFunctionType.Relu)
                for half in range(2):
                    sl = slice(half * HO, (half + 1) * HO)
                    nc.scalar.activation(h_sb[1][:, sl], ph[1][:, sl],
                                         mybir.ActivationFunctionType.Relu)

                # --- aggregation: OT[kd, n] = sum_m h[m, kd] * adjT[m, n]
                # k -> (q = k%2 bank, s = (k//2)%2 partition half,
                #       t = k//4 column half); within each t, the 4 open
                # accumulation groups occupy distinct (bank, partition-range).
                po = [ps_o.tile([P, OUT_DIM], FP32, tag="pso", name=f"po{b}_{q}")
                      for q in range(2)]

                def a_sl(k, c):
                    src, kk = (aS, k) if k < 4 else (aC, k - 4)
                    fo = (kk * NC2 + c) * N
                    return src[:, fo:fo + N]

                for t in range(2):
                    for c in range(NC2):
                        for kk in range(4):
                            k = 4 * t + kk
                            q, s = k % 2, (k // 2) % 2
                            nc.tensor.matmul(
                                po[q][s * DK:(s + 1) * DK,
                                      t * 2 * P:(t + 1) * 2 * P],
                                h_sb[c][:, k * DK:(k + 1) * DK],
                                a_sl(k, c),
                                start=(c == 0), stop=(c == NC2 - 1))

                # --- drain accumulators (cast bf16) + store
                for q in range(2):
                    ot = p_out.tile([P, OUT_DIM], BF16, tag="o", name=f"o{b}_{q}")
                    if q == 0:
                        nc.vector.tensor_copy(ot[:], po[q][:])
                    else:
                        nc.scalar.copy(ot[:], po[q][:])
                    if b < BPC - 2:
                        nc.gpsimd.dma_start(out=out_ap[b, q], in_=ot[:])
                    elif q == 0:
                        nc.sync.dma_start(out=out_ap[b, q], in_=ot[:])
                    else:
                        nc.scalar.dma_start(out=out_ap[b, q], in_=ot[:])

                # prefetch last: DMA issues trail this batch's compute in
                # program order (no head-of-line blocking)
                if b + PF < BPC:
                    prefetch(b + PF)

    nc.compile()
    return nc


def _get_nc():
    if "nc" not in _compiled:
        _compiled["nc"] = _build()
    return _compiled["nc"]


def _pack_inputs(inputs):
    node_feats = np.asarray(inputs["node_feats"])
    adj = np.asarray(inputs["adj"])
    weight = np.asarray(inputs["weight"])
    bias = np.asarray(inputs["bias"])

    # x^T packed [B, p, (ic, n)] with i = ic*128 + p
    xt = node_feats.swapaxes(1, 2).astype(BF)          # [B, IN, N]
    xt = np.ascontiguousarray(
        xt.reshape(B, IC4, P, N).transpose(0, 2, 1, 3)).reshape(B, P, IC4 * N)

    # adj^T packed [B, p, (k, c, n)] with m = c*128 + p
    adjt = adj.swapaxes(2, 3).astype(BF)               # [B, K, N(m), N(n)]
    adjt = np.ascontiguousarray(
        adjt.reshape(B, K, NC2, P, N).transpose(0, 3, 1, 2, 4)
    ).reshape(B, P, K * NC2 * N)

    w = weight.astype(BF).reshape(IC4, P, OUT_DIM).transpose(1, 0, 2)
    w = np.ascontiguousarray(w).reshape(P, IC4 * OUT_DIM)
    bias_bf = bias.astype(BF)
    return xt, adjt, w, bias_bf


def _run(inputs, trace=False, trace_cores=None):
    nc = _get_nc()
    xt, adjt, w, bias_bf = _pack_inputs(inputs)
    in_maps = []
    for c in range(N_CORES):
        sl = slice(c * BPC, (c + 1) * BPC)
        in_maps.append({
            "xt": xt[sl],
            "adjp": adjt[sl],
            "w": w,
            "bias": bias_bf,
        })
    res = run_bass_kernel_spmd(
        nc, in_maps, core_ids=list(range(N_CORES)),
        trace=trace, trace_cores=trace_cores)
    raw = np.concatenate(
        [np.asarray(res.results[c]["out"]) for c in range(N_CORES)], axis=0)
    # raw [B, q, p, col]: o = t*256 + s*128 + q*64 + d, n = col%256
    # with s = p//64, d = p%64, t = col//256.
    v = raw.astype(np.float32).reshape(B, 2, 2, DK, 2, N)  # b, q, s, d, t, n
    out = v.transpose(0, 5, 4, 2, 1, 3).reshape(B, N, OUT_DIM)
    return np.ascontiguousarray(out), res


def kernel(**inputs) -> np.ndarray:
    return _run(inputs, trace=False)[0]
